# revision 26
# baseline (speedup 1.0000x reference)
"""Trainium2 Bass kernel for nn_ALNet (adaptive linear network forward).

Math: vals = x @ W + b  ([65536,256] @ [256,128] + [128]), then a 7-level
alternating min/max pairwise tree over the 128 leaf columns -> [B, 1].

v5 strategy (8 NeuronCores, data-parallel over batch; per-core shard 8192):
  x-stationary matmuls produce psum ALREADY in [batch, leaf] orientation, so
  the whole [leaf,batch] eviction + PE-transpose stage of earlier versions
  disappears.  Per [128-batch x 128-leaf] tile: LDW(x-block)+MM(N=128,
  rhs=W-half) twice -- measured 67 ns per LDW+MM pair (FWL hides the load).
  The bias is seeded into psum first via K=1 matmuls (ones-row stationary,
  b-tiled rhs, N=512), so no per-free bias op is ever needed downstream.

  Per 1024-col unit (2 psum banks): 2 bias seeds + 16 tile matmuls ->
  ACT stages the B-halves (leaf cols 64:128 of each block, strided) to SBUF
  -> DVE L1 = min(psum A-halves, staged B) straight from psum (only one
  psum operand allowed per DVE op) -> tree levels 2..3 per 2048-span on DVE
  -> levels 4..7 once globally in 4 wide ops.

  DMA: chunk-major host pack [h0 | h1] per chunk so every chunk DMA reads
  8KB contiguous per-partition runs (~345 GB/s effective; 4KB runs cap
  ~290).  Sync ring carries chunks 0,1,3; scalar (HWDGE starts ~3.2us late
  behind the ACT table load) carries W+b then chunks 2,4.
"""

import numpy as np

try:
    import concourse.bass as bass
except ImportError:  # pragma: no cover
    import sys

    sys.path.insert(0, "/opt/trn_rl_repo")
    import concourse.bass as bass

import concourse.mybir as mybir
import concourse.tile as tile
from concourse import bacc
from concourse.bass_utils import run_bass_kernel_spmd

F32 = mybir.dt.float32
F16 = mybir.dt.float16

B, F, NL = 65536, 256, 128
NCORES = 8
BS = B // NCORES  # 8192 batch rows per core

UNIT = 1024  # pipeline granularity (2 psum banks)
NUNIT = BS // UNIT  # 8
# x chunks (col0, width): one 1024-col chunk per unit so every unit gets its
# own completion semaphore ~1.5us after its bytes (fat chunks make the whole
# pipeline wait a single late semaphore)
CHUNKS = [(1024 * c, 1024) for c in range(8)]
SYNC_CHUNKS = {0, 1, 2, 3}  # units 4-7 (+wb) ride the scalar ring

# wb layout: [W0 (128) | W1 (128) | row0: ones (128) | row0: b4 (512)]
WB_W0, WB_W1, WB_ONES, WB_B4, WB_COLS = 0, 128, 256, 384, 896

# Tree ops, deepest level first (palindrome list: min,max,min,max,min,max,min)
_TREE_OPS = [
    mybir.AluOpType.min if i % 2 == 0 else mybir.AluOpType.max for i in range(7)
]


def _bitrev7_perm() -> np.ndarray:
    perm = np.zeros(NL, dtype=np.int64)
    for p in range(NL):
        r = 0
        for k in range(7):
            r |= ((p >> k) & 1) << (6 - k)
        perm[p] = r
    return perm


def _chunk_of(col):
    for c, (c0, w) in enumerate(CHUNKS):
        if c0 <= col < c0 + w:
            return c, c0, w
    raise ValueError(col)


def build_nc(bs: int = BS):
    nc = bacc.Bacc(None)
    # xpk: chunk-major pack, chunk c -> pack cols [2*c0, 2*c0+2w) as [h0 | h1]
    xpkd = nc.declare_dram_parameter("xpk", [128, 2 * bs], F16, isOutput=False)
    wbd = nc.declare_dram_parameter("wb", [128, WB_COLS], F16, isOutput=False)
    ncols = bs // 128  # 64
    out = nc.declare_dram_parameter("out", [128, ncols], F32, isOutput=True)

    with tile.TileContext(nc, pool_alloc_mode="queue") as tc:
        with (
            tc.tile_pool(name="xin", bufs=1) as xpool,
            tc.tile_pool(name="psum", bufs=4, space=bass.MemorySpace.PSUM) as ppool,
            tc.tile_pool(name="sb", bufs=1) as spool,
        ):
            xpk = xpool.tile([128, 2 * bs], F16, tag="xpk")
            wb = xpool.tile([128, WB_COLS], F16, tag="wb")

            nc.scalar.dma_start(out=wb[:], in_=wbd[:])
            for c, (c0, w) in enumerate(CHUNKS):
                ring = nc.sync if c in SYNC_CHUNKS else nc.scalar
                s = slice(2 * c0, 2 * c0 + 2 * w)
                ring.dma_start(out=xpk[:, s], in_=xpkd[:, s])

            w0t = wb[:, WB_W0 : WB_W0 + 128]
            w1t = wb[:, WB_W1 : WB_W1 + 128]
            onesr = wb[0:1, WB_ONES : WB_ONES + 128]
            b4r = wb[0:1, WB_B4 : WB_B4 + 512]

            def xblk(col, half):
                # [128, 128] stationary block for batch cols [col, col+128)
                c, c0, w = _chunk_of(col)
                off = 2 * c0 + w * half + (col - c0)
                return xpk[:, off : off + 128]

            # SBUF intermediates
            vtb = spool.tile([128, bs // 2], F16, tag="vtb")  # staged B-halves
            l1 = spool.tile([128, bs // 2], F16, tag="l1")
            lvl_tiles = []
            w = 32
            n = bs // 4
            while w >= 2:
                lvl_tiles.append(
                    spool.tile([128, n], F16, tag=f"lv{w}", name=f"lv{w}")
                )
                w //= 2
                n //= 2
            ost = spool.tile([128, ncols], F32, tag="ost")

            # PE warmup: garbage matmuls with no input deps
            garb = spool.tile([128, 512], F16, tag="garb")
            nc.vector.memset(garb[:], 0.0)
            pss = {}
            for u in range(NUNIT):
                pss[u] = ppool.tile([128, UNIT], F32, tag="ps", name=f"ps_{u}")
            for i in range(6):
                nc.tensor.matmul(
                    pss[0][:, 0:512], garb[:, 0:128], garb[:],
                    start=True, stop=True,
                )

            def tree_low(col0, ncols_span):
                # levels 2..3 for batch cols [col0, col0+ncols_span) on DVE
                cur = l1[:, col0 // 2 : (col0 + ncols_span) // 2]
                w = 32
                for lvl in range(1, 3):
                    r = cur.rearrange("p (blk two h) -> p blk two h", two=2, h=w)
                    base = lvl_tiles[lvl - 1]
                    nxt = base[
                        :, (col0 // 128) * w : ((col0 + ncols_span) // 128) * w
                    ]
                    nc.vector.tensor_tensor(
                        out=nxt.rearrange("p (blk h) -> p blk h", h=w),
                        in0=r[:, :, 0, :], in1=r[:, :, 1, :],
                        op=_TREE_OPS[lvl],
                    )
                    cur = nxt
                    w //= 2

            def tree_high(half):
                # levels 4..7 over one half of the shard (keeps the final
                # tail to one half's worth of small ops)
                hw = bs // 16  # lv16 cols per half
                cur = lvl_tiles[1][:, half * hw : (half + 1) * hw]
                w = 8
                for lvl in range(3, 7):
                    r = cur.rearrange("p (blk two h) -> p blk two h", two=2, h=w)
                    if lvl < 6:
                        n = hw // 16 * w
                        nxt = lvl_tiles[lvl - 1][:, half * n : (half + 1) * n]
                        outap = nxt.rearrange("p (blk h) -> p blk h", h=w)
                    else:
                        nxt = None
                        outap = ost[
                            :, half * (ncols // 2) : (half + 1) * (ncols // 2)
                        ].rearrange("p (blk h) -> p blk h", h=1)
                    nc.vector.tensor_tensor(
                        out=outap, in0=r[:, :, 0, :], in1=r[:, :, 1, :],
                        op=_TREE_OPS[lvl],
                    )
                    cur = nxt
                    w //= 2

            # main pipeline over 1024-col units
            for u in range(NUNIT):
                ps = pss[u]
                c0 = u * UNIT
                # bias seeds: psum[b, l] = b4[l] for every batch partition
                for bank in range(2):
                    nc.tensor.matmul(
                        ps[:, bass.ts(bank, 512)], onesr, b4r,
                        start=True, stop=False,
                    )
                # 8 [128b x 128l] tiles, two K-halves each
                for j in range(8):
                    col = c0 + j * 128
                    reg = ps[:, j * 128 : j * 128 + 128]
                    nc.tensor.matmul(
                        reg, xblk(col, 0), w0t, start=False, stop=False
                    )
                    nc.tensor.matmul(
                        reg, xblk(col, 1), w1t, start=False, stop=True
                    )
                l1_s = l1[:, c0 // 2 : (c0 + UNIT) // 2].rearrange(
                    "p (blk h) -> p blk h", h=64
                )
                if u == NUNIT - 1:
                    # last unit: single fused reduce (pair axis innermost)
                    # skips the ACT staging hop on the critical tail
                    nc.vector.tensor_reduce(
                        out=l1_s,
                        in_=ps[:].rearrange(
                            "p (blk two h) -> p blk h two", two=2, h=64
                        ),
                        axis=mybir.AxisListType.X,
                        op=_TREE_OPS[0],
                    )
                else:
                    # stage B-halves (leaf cols 64:128 per block) on ACT,
                    # then L1 = min(psum A-halves, staged B) on DVE (only
                    # one psum operand allowed per DVE op)
                    rr = ps[:].rearrange(
                        "p (blk two h) -> p blk two h", two=2, h=64
                    )
                    vtb_s = vtb[:, c0 // 2 : (c0 + UNIT) // 2].rearrange(
                        "p (blk h) -> p blk h", h=64
                    )
                    nc.scalar.activation(
                        vtb_s, rr[:, :, 1, :],
                        mybir.ActivationFunctionType.Copy,
                    )
                    nc.vector.tensor_tensor(
                        out=l1_s, in0=rr[:, :, 0, :], in1=vtb_s,
                        op=_TREE_OPS[0],
                    )
                if u % 2 == 1:
                    tree_low((u - 1) * UNIT, 2 * UNIT)
                    if u == 3:
                        tree_high(0)

            tree_high(1)
            nc.sync.dma_start(out=out[:], in_=ost[:])

    nc.compile()
    return nc


_NC_CACHE: dict = {}


def _get_nc(bs=BS):
    if bs not in _NC_CACHE:
        _NC_CACHE[bs] = build_nc(bs)
    return _NC_CACHE[bs]


def prep_inputs(x: np.ndarray, W: np.ndarray, b: np.ndarray) -> list[dict]:
    perm = _bitrev7_perm()
    Wp = np.ascontiguousarray(W[:, perm]).astype(np.float16)
    bh = b[perm].astype(np.float16)
    x = np.asarray(x, dtype=np.float32)
    wb = np.zeros((128, WB_COLS), dtype=np.float16)
    wb[:, WB_W0 : WB_W0 + 128] = Wp[0:128, :]
    wb[:, WB_W1 : WB_W1 + 128] = Wp[128:256, :]
    wb[0, WB_ONES : WB_ONES + 128] = 1.0
    wb[0, WB_B4 : WB_B4 + 512] = np.tile(bh, 4)
    in_maps = []
    for i in range(NCORES):
        xi = x[i * BS : (i + 1) * BS, :].astype(np.float16)  # [8192, 256]
        xT = xi.T  # [256, 8192]
        xpk = np.empty((128, 2 * BS), dtype=np.float16)
        for c0, w in CHUNKS:
            xpk[:, 2 * c0 : 2 * c0 + w] = xT[0:128, c0 : c0 + w]
            xpk[:, 2 * c0 + w : 2 * c0 + 2 * w] = xT[128:256, c0 : c0 + w]
        in_maps.append({"xpk": xpk, "wb": wb})
    return in_maps


def gather_outputs(results: list[dict]) -> np.ndarray:
    shards = []
    for i in range(NCORES):
        o = np.asarray(results[i]["out"])  # [128, BS//128]; o[p, c] = row 128c+p
        shards.append(o.T.reshape(BS))
    return np.concatenate(shards).reshape(B, 1).astype(np.float32)


def _setup_tracing():
    """Install the antenv.axon_hooks NTFF-profile shim (missing from this
    image) and neuter the artifact upload so traced runs stay local."""
    import sys as _sys
    import types

    import concourse.bass_utils as bu

    bu.upload_artifacts = lambda tmpdir: tmpdir
    try:
        from antenv.axon_hooks import get_axon_ntff_profile_hook  # noqa: F401

        return
    except ImportError:
        pass
    import antenv

    m = types.ModuleType("antenv.axon_hooks")
    _state = {"hook": None}
    m.set_axon_ntff_profile_hook = lambda h: _state.__setitem__("hook", h)
    m.get_axon_ntff_profile_hook = lambda: _state["hook"]
    _sys.modules["antenv.axon_hooks"] = m
    antenv.axon_hooks = m
    try:
        from trn_agent_boot.trn_boot import _ntff_profile_via_ctypes

        hook = _ntff_profile_via_ctypes("/opt/axon/libaxon_pjrt.so")
        if hook is not None:
            m.set_axon_ntff_profile_hook(hook)
    except Exception as e:  # pragma: no cover
        print("ntff hook install failed:", e)


def run_on_hw(x, W, b, trace: bool = False, **kwargs):
    if trace:
        _setup_tracing()
    nc = _get_nc()
    in_maps = prep_inputs(np.asarray(x), np.asarray(W), np.asarray(b))
    return run_bass_kernel_spmd(
        nc, in_maps, core_ids=list(range(NCORES)), trace=trace, **kwargs
    )


def kernel(x: np.ndarray, W: np.ndarray, b: np.ndarray) -> np.ndarray:
    res = run_on_hw(x, W, b, trace=False)
    return gather_outputs(res.results)
